# revision 1
# baseline (speedup 1.0000x reference)
"""Self-contained Trainium2 Bass kernel for nn_EntAttentionLayer.

Sharding: 8 cores = (batch 4) x (sequence half 2), no collectives.
Each core computes its [1024 tokens, 1024 hid] slice of the output
end-to-end: self-attention (banded mask) -> cross-attention to tag
embeddings -> FFN, each with residual + LayerNorm.

Device layout: activations kept transposed [hid(part), tok(free)].
  - scores computed transposed S^T[k, q] = (K^T slice).T @ (Q^T slice)
  - band mask added via identity-matmul accumulation into PSUM
  - softmax without max subtraction (scores are O(1) for this model)
  - sum_k exp folded into PV matmul via a ones-column appended to V
  - LayerNorm over partitions via ones-vector matmuls
  - [1,N] -> [128,N] broadcasts via DRAM round-trip DMA
Per-core inputs are staged with the sequence ROTATED by half*1024 so all
8 cores run the identical program (band tiles are core-local data).
Matmul operands in bf16 (fp32 PSUM accumulate); residual/LN math in fp32.
"""

import sys

for _p in ("/opt/trn_rl_repo",):
    if _p not in sys.path:
        sys.path.insert(0, _p)

import numpy as np
import ml_dtypes

import concourse.bacc as bacc
import concourse.mybir as mybir
import concourse.tile as tile
from concourse.tile import add_dep_helper
from concourse.bass_utils import run_bass_kernel_spmd

BF = ml_dtypes.bfloat16
fp32 = mybir.dt.float32
bf16 = mybir.dt.bfloat16

H = 1024          # hidden
S = 2048          # full sequence
QL = 1024         # per-core query tokens
FFN = 4096
NH, HD = 16, 64
P = 128
HT = H // P       # 8 hid tiles
ST = S // P       # 16 seq tiles
QN = QL // 512    # 2 q blocks of 512
EPS = 1e-12

# pvec column offsets (per-partition param pack, [128, PCOLS] fp32)
QB8, KB, SOB, SLG, SLB = 0, 8, 16, 24, 32
CQB8, CKB, COB, CLG, CLB = 40, 48, 56, 64, 72
IB, OB, OLG, OLB = 80, 112, 120, 128
ONECOL = 136  # column of fp32 ones (LN mean matmul lhsT)
PCOLS = 137

_CACHE = {}


def _band_needed(er):
    """(kt, qn) pairs, in local (rotated) coords, where the band tile can be
    nonzero for either half. Core-independent."""
    out = []
    for kt in range(ST):
        for qn in range(QN):
            lo, hi = qn * 512 - er, qn * 512 + 511 + er
            k0, k1 = kt * P, kt * P + 127
            if (k0 <= hi and k1 >= lo) or (k0 - S <= hi and k1 - S >= lo):
                out.append((kt, qn))
    return out


def _build(er):
    """Build + bacc-compile the per-core program. er = ent_range (>=0)."""
    band_kq = _band_needed(er) if er > 0 else []
    nc = bacc.Bacc()

    # ---- DRAM I/O ----
    xT = nc.dram_tensor("xT", [H, S], bf16, kind="ExternalInput")
    xh = nc.dram_tensor("xh", [H, QL], fp32, kind="ExternalInput")
    w_sq = nc.dram_tensor("w_sq", [H, H], bf16, kind="ExternalInput")
    w_sk = nc.dram_tensor("w_sk", [H, H], bf16, kind="ExternalInput")
    w_sv = nc.dram_tensor("w_sv", [H, H], bf16, kind="ExternalInput")
    w_so = nc.dram_tensor("w_so", [H, H], bf16, kind="ExternalInput")
    w_cq = nc.dram_tensor("w_cq", [H, H], bf16, kind="ExternalInput")
    w_ck = nc.dram_tensor("w_ck", [H, H], bf16, kind="ExternalInput")
    w_cv = nc.dram_tensor("w_cv", [H, H], bf16, kind="ExternalInput")
    w_co = nc.dram_tensor("w_co", [H, H], bf16, kind="ExternalInput")
    w_i = nc.dram_tensor("w_i", [H, FFN], bf16, kind="ExternalInput")
    w_o = nc.dram_tensor("w_o", [FFN, H], bf16, kind="ExternalInput")
    pvec = nc.dram_tensor("pvec", [P, PCOLS], fp32, kind="ExternalInput")
    tagsT = nc.dram_tensor("tagsT", [H, 64], bf16, kind="ExternalInput")
    ident_d = nc.dram_tensor("ident", [P, P], bf16, kind="ExternalInput")
    ones_d = nc.dram_tensor("ones", [P, 1], bf16, kind="ExternalInput")
    nb = max(len(band_kq), 1)
    band_d = nc.dram_tensor("band", [nb, P, 512], bf16, kind="ExternalInput")
    yT = nc.dram_tensor("yT", [H, QL], fp32, kind="ExternalOutput")

    T = 50  # tags count (tagsT padded to 64 cols)
    FT = FFN // P  # 32
    FC = 8         # FFN m-tiles per chunk (4 chunks)

    with tile.TileContext(nc) as tc:
        with tc.tile_pool(name="p1", bufs=1) as p1, \
             tc.tile_pool(name="p2", bufs=2) as p2, \
             tc.tile_pool(name="p3", bufs=3) as p3, \
             tc.tile_pool(name="p4", bufs=4) as p4, \
             tc.tile_pool(name="psA", bufs=4, space="PSUM") as psA, \
             tc.tile_pool(name="psC", bufs=4, space="PSUM") as psC, \
             tc.tile_pool(name="dram", bufs=1, space="DRAM") as dpool, \
             tc.tile_pool(name="dscr", bufs=4, space="DRAM") as dscr:

            # ---- constants ----
            identt = p1.tile([P, P], bf16, tag="ident")
            nc.sync.dma_start(out=identt[:], in_=ident_d[:, :])
            onest = p1.tile([P, 1], bf16, tag="ones")
            nc.sync.dma_start(out=onest[:], in_=ones_d[:, :])
            pv = p1.tile([P, PCOLS], fp32, tag="pvec")
            nc.sync.dma_start(out=pv[:], in_=pvec[:, :])
            tg = p1.tile([P, HT, 64], bf16, tag="tags")
            nc.sync.dma_start(out=tg[:], in_=tagsT[:, :].rearrange("(t p) c -> p t c", p=P))
            eps_t = p1.tile([1, 1], fp32, tag="eps")
            nc.vector.memset(eps_t[:], EPS)

            def col(c):
                return pv[:, c:c + 1]

            # work: fp32 [128, 8, 1024]; x^T(half) -> t1 -> a -> t2 -> c -> t3 -> y
            # (loaded later, just before the o-proj residual needs it)
            work = p1.tile([P, HT, QL], fp32, tag="work")

            # DRAM scratch for K^T and V_aug
            kT_d = dpool.tile([H, S], bf16, name="kT_d")
            vaug = dpool.tile([S, NH, 66], bf16, name="vaug_d")

            def load_w(handle, n_m=HT):
                # split per m-block so the first matmuls start after 1/8 of the DMA
                wt = p2.tile([P, HT, n_m * P], bf16, tag="w", bufs=2, name="wt")
                for m in range(n_m):
                    nc.sync.dma_start(out=wt[:, :, m * P:(m + 1) * P],
                                      in_=handle[:, m * P:(m + 1) * P]
                                      .rearrange("(t p) m -> p t m", p=P))
                return wt

            # ---------- Phase 1: Q^T = ((x_half @ wq) + bq)/8 ----------
            q_sb = p1.tile([P, HT, QL], bf16, tag="qT")
            xqs = []
            for qn in range(QN):
                xq = p2.tile([P, HT, 512], bf16, tag="xs", bufs=2, name="xq")
                xqs.append(xq)
            w = p2.tile([P, HT, HT * P], bf16, tag="w", bufs=2, name="wt")
            nc.sync.dma_start(out=w[:, :, 0:P], in_=w_sq[:, 0:P]
                              .rearrange("(t p) m -> p t m", p=P))
            nc.sync.dma_start(out=xqs[0][:], in_=xT[:, 0:512]
                              .rearrange("(t p) q -> p t q", p=P))
            for m in range(1, HT):
                nc.sync.dma_start(out=w[:, :, m * P:(m + 1) * P],
                                  in_=w_sq[:, m * P:(m + 1) * P]
                                  .rearrange("(t p) m -> p t m", p=P))
            nc.sync.dma_start(out=xqs[1][:], in_=xT[:, 512:1024]
                              .rearrange("(t p) q -> p t q", p=P))
            q_anchor = None
            for qn in range(QN):
                xq = xqs[qn]
                for m in range(HT):
                    ps = psA.tile([P, 512], fp32, tag="mm", name="psq")
                    for kt in range(HT):
                        mm = nc.tensor.matmul(ps[:], w[:, kt, m * P:(m + 1) * P],
                                              xq[:, kt, :],
                                              start=(kt == 0), stop=(kt == HT - 1))
                        if q_anchor is None:
                            q_anchor = mm
                    nc.scalar.activation(out=q_sb[:, m, qn * 512:(qn + 1) * 512], in_=ps[:],
                                         func=mybir.ActivationFunctionType.Identity,
                                         bias=col(QB8 + m), scale=0.125)

            # ---------- Phase 2: K^T = x @ wk + bk -> DRAM ----------
            w = load_w(w_sk)
            k_anchor = None
            for sn in range(S // 512):
                xk = p2.tile([P, HT, 512], bf16, tag="xs", bufs=2, name="xk")
                nc.sync.dma_start(out=xk[:], in_=xT[:, sn * 512:(sn + 1) * 512]
                                  .rearrange("(t p) q -> p t q", p=P))
                for m in range(HT):
                    ps = psA.tile([P, 512], fp32, tag="mm", name="psk")
                    for kt in range(HT):
                        mm = nc.tensor.matmul(ps[:], w[:, kt, m * P:(m + 1) * P],
                                              xk[:, kt, :],
                                              start=(kt == 0), stop=(kt == HT - 1))
                        if k_anchor is None:
                            k_anchor = mm
                        k_last = mm
                    kt_t = p2.tile([P, 512], bf16, tag="ktmp", name="kt_t")
                    nc.scalar.activation(out=kt_t[:], in_=ps[:],
                                         func=mybir.ActivationFunctionType.Identity,
                                         bias=col(KB + m), scale=1.0)
                    nc.sync.dma_start(out=kT_d[m * P:(m + 1) * P, sn * 512:(sn + 1) * 512],
                                      in_=kt_t[:])

            # ---------- Phase 3: V natural + ones col -> DRAM ----------
            # band tiles load here (gpsimd queue), ready before attention;
            # held back behind Q-proj so it can't crowd the startup DMAs
            band_sb = None
            if band_kq:
                band_sb = p1.tile([P, len(band_kq), 512], bf16, tag="band", name="band_sb")
                band_dma = nc.gpsimd.dma_start(out=band_sb[:],
                                               in_=band_d[:, :, :].rearrange("t p c -> p t c"))
                add_dep_helper(band_dma.ins, k_anchor.ins, sync=True,
                               reason="delay band load past startup")
            band_idx = {kq: i for i, kq in enumerate(band_kq)}

            w = load_w(w_sv)
            v_anchor = None
            for tt in range(ST):
                xv = p2.tile([P, HT, P], bf16, tag="xs", bufs=2, name="xv")
                nc.sync.dma_start(out=xv[:], in_=xT[:, tt * P:(tt + 1) * P]
                                  .rearrange("(t p) q -> p t q", p=P))
                vt = p2.tile([P, NH, 66], bf16, tag="vv", bufs=1, name="vt")
                for ds in range(2):
                    ps = psA.tile([P, 512], fp32, tag="mm", name="psv")
                    for kt in range(HT):
                        mm = nc.tensor.matmul(ps[:], xv[:, kt, :],
                                              w[:, kt, ds * 512:(ds + 1) * 512],
                                              start=(kt == 0), stop=(kt == HT - 1))
                        if v_anchor is None:
                            v_anchor = mm
                        v_last = mm
                    nc.vector.tensor_copy(
                        out=vt[:, ds * 8:(ds + 1) * 8, 0:64],
                        in_=ps[:].rearrange("p (h c) -> p h c", c=64))
                nc.vector.memset(vt[:, :, 64:66], 1.0)
                nc.sync.dma_start(out=vaug[tt * P:(tt + 1) * P, :, :], in_=vt[:])

            # ---------- Phase 4: self-attention ----------
            # x residual loads during attention (anchored below), in halves
            work_dmas = [nc.gpsimd.dma_start(out=work[:, 0:4, :],
                                             in_=xh[0:512, :].rearrange("(t p) q -> p t q", p=P)),
                         nc.gpsimd.dma_start(out=work[:, 4:8, :],
                                             in_=xh[512:1024, :].rearrange("(t p) q -> p t q", p=P))]
            ctx_sb = p1.tile([P, HT, QL], bf16, tag="ctx")
            att_anchors = {}
            for a in range(NH // 2):
                kp = p2.tile([P, S], bf16, tag="kpair", name="kp")
                vp = p2.tile([P, ST, 2, 66], bf16, tag="vp", name="vp")
                # quarter-split so the first QK/PV start after 1/4 transfer
                for c in range(4):
                    nc.sync.dma_start(out=kp[:, c * 512:(c + 1) * 512],
                                      in_=kT_d[a * P:(a + 1) * P, c * 512:(c + 1) * 512])
                    nc.sync.dma_start(out=vp[:, c * 4:(c + 1) * 4, :, :],
                                      in_=vaug[c * 512:(c + 1) * 512, 2 * a:2 * a + 2, :]
                                      .rearrange("(kt p) h c -> p kt h c", p=P))
                for qn in range(QN):
                    cps = [psC.tile([65, 512], fp32, tag="ctx", name=f"ctxps{i}")
                           for i in range(2)]
                    for kt in range(ST):
                        for hh in range(2):
                            sp = psA.tile([P, 512], fp32, tag="mm", name="sps")
                            if (a, qn, kt, hh) in ((1, 0, 0, 0), (2, 0, 0, 0)):
                                att_anchors[a] = None  # filled after matmul below
                            has_band = (kt, qn) in band_idx
                            mm = nc.tensor.matmul(
                                sp[:],
                                kp[hh * 64:(hh + 1) * 64, kt * P:(kt + 1) * P],
                                q_sb[hh * 64:(hh + 1) * 64, a, qn * 512:(qn + 1) * 512],
                                start=True, stop=not has_band)
                            if a in att_anchors and att_anchors[a] is None:
                                att_anchors[a] = mm
                            if has_band:
                                nc.tensor.matmul(sp[:], identt[:],
                                                 band_sb[:, band_idx[(kt, qn)], :],
                                                 start=False, stop=True)
                            pt = p4.tile([P, 512], bf16, tag="ptile", bufs=4, name="pt")
                            nc.scalar.activation(out=pt[:], in_=sp[:],
                                                 func=mybir.ActivationFunctionType.Exp)
                            nc.tensor.matmul(cps[hh][:], vp[:, kt, hh, 0:65], pt[:],
                                             start=(kt == 0), stop=(kt == ST - 1))
                    for hh in range(2):
                        cp = cps[hh]
                        rec = p3.tile([1, 512], fp32, tag="rows", bufs=2, name="rec")
                        nc.vector.reciprocal(out=rec[:], in_=cp[64:65, :])
                        scr = dscr.tile([1, 512], fp32, name="scr_a")
                        nc.sync.dma_start(out=scr[:, :], in_=rec[:])
                        bc = p2.tile([64, 512], fp32, tag="bc64", name="bca")
                        nc.sync.dma_start(out=bc[:], in_=scr[0:1, :].partition_broadcast(64))
                        nc.vector.tensor_mul(
                            out=ctx_sb[hh * 64:(hh + 1) * 64, a, qn * 512:(qn + 1) * 512],
                            in0=cp[0:64, :], in1=bc[:])

            for i, (a, mm) in enumerate(sorted(att_anchors.items())):
                add_dep_helper(work_dmas[i].ins, mm.ins, sync=True,
                               reason="residual load rides mid-attention")

            # ---------- residual-add + LayerNorm helpers (transposed) ----------
            def layer_norm(gcol, bcol, out_bf=None):
                """work holds t (fp32). Normalize in place; optional bf16 copy."""
                for qn in range(QN):
                    qs = slice(qn * 512, (qn + 1) * 512)
                    mean_ps = psC.tile([1, 512], fp32, tag="ctx", name="mean_ps")
                    sq_ps = psC.tile([1, 512], fp32, tag="ctx", name="sq_ps")
                    for kt in range(HT):
                        # mean directly from fp32 work (no copy step on the chain)
                        nc.tensor.matmul(mean_ps[:], col(ONECOL), work[:, kt, qs],
                                         start=(kt == 0), stop=(kt == HT - 1))
                        sb_ = p2.tile([P, 512], bf16, tag="sqb", name="sb_")
                        nc.scalar.activation(out=sb_[:], in_=work[:, kt, qs],
                                             func=mybir.ActivationFunctionType.Square)
                        nc.tensor.matmul(sq_ps[:], onest[:], sb_[:],
                                         start=(kt == 0), stop=(kt == HT - 1))
                    negmean = p3.tile([1, 512], fp32, tag="rows", bufs=2, name="negmean")
                    nc.scalar.mul(out=negmean[:], in_=mean_ps[:], mul=-1.0 / H)
                    msq = p3.tile([1, 512], fp32, tag="rows", bufs=2, name="msq")
                    nc.scalar.mul(out=msq[:], in_=sq_ps[:], mul=1.0 / H)
                    scr = dscr.tile([2, 512], fp32, name="scr_ln")
                    nc.sync.dma_start(out=scr[0:1, :], in_=negmean[:])
                    # negmean shipped; square it in place, then var/std/inv in msq
                    nc.vector.tensor_mul(out=negmean[:], in0=negmean[:], in1=negmean[:])
                    nc.vector.tensor_sub(out=msq[:], in0=msq[:], in1=negmean[:])
                    nc.scalar.activation(out=msq[:], in_=msq[:],
                                         func=mybir.ActivationFunctionType.Sqrt,
                                         bias=eps_t[:])
                    nc.vector.reciprocal(out=msq[:], in_=msq[:])
                    nc.sync.dma_start(out=scr[1:2, :], in_=msq[:])
                    nm_bc = p2.tile([P, 512], fp32, tag="bc", name="nm_bc")
                    nc.sync.dma_start(out=nm_bc[:], in_=scr[0:1, :].partition_broadcast(P))
                    iv_bc = p2.tile([P, 512], fp32, tag="bc", name="iv_bc")
                    nc.sync.dma_start(out=iv_bc[:], in_=scr[1:2, :].partition_broadcast(P))
                    for j in range(HT):
                        nc.vector.tensor_add(out=work[:, j, qs], in0=work[:, j, qs],
                                             in1=nm_bc[:])
                        nc.vector.tensor_mul(out=work[:, j, qs], in0=work[:, j, qs],
                                             in1=iv_bc[:])
                        # affine (x*g + b) on ACT: frees DVE, per-partition APs
                        nc.scalar.activation(out=work[:, j, qs], in_=work[:, j, qs],
                                             func=mybir.ActivationFunctionType.Identity,
                                             bias=col(bcol + j), scale=col(gcol + j))
                        if out_bf is not None:
                            nc.vector.tensor_copy(out=out_bf[:, j, qs],
                                                  in_=work[:, j, qs])

            def proj_add_residual(w, rhs, bcol):
                """work <- (proj of rhs via w) + bias + work, per [m, qn] tile."""
                nkt = rhs.shape[1]
                for m in range(HT):
                    for qn in range(QN):
                        qs = slice(qn * 512, (qn + 1) * 512)
                        ps = psA.tile([P, 512], fp32, tag="mm", name="pso")
                        for kt in range(nkt):
                            nc.tensor.matmul(ps[:], w[:, kt, m * P:(m + 1) * P],
                                             rhs[:, kt, qs],
                                             start=(kt == 0), stop=(kt == nkt - 1))
                        nc.vector.scalar_tensor_tensor(
                            out=work[:, m, qs], in0=ps[:], scalar=col(bcol + m),
                            in1=work[:, m, qs],
                            op0=mybir.AluOpType.add, op1=mybir.AluOpType.add)

            # ---------- Phase 5: self out-proj + residual + LN1 ----------
            w = load_w(w_so)
            proj_add_residual(w, ctx_sb, SOB)
            a_bf = p1.tile([P, HT, QL], bf16, tag="act_bf")
            layer_norm(SLG, SLB, out_bf=a_bf)

            # ---------- Phase 6: cross-attention ----------
            w = load_w(w_ck)
            kc = p1.tile([P, HT, T], bf16, tag="kc")
            for m in range(HT):
                ps = psA.tile([P, T], fp32, tag="mm", name="pskc")
                for kt in range(HT):
                    nc.tensor.matmul(ps[:], w[:, kt, m * P:(m + 1) * P], tg[:, kt, 0:T],
                                     start=(kt == 0), stop=(kt == HT - 1))
                nc.scalar.activation(out=kc[:, m, :], in_=ps[:],
                                     func=mybir.ActivationFunctionType.Identity,
                                     bias=col(CKB + m), scale=1.0)

            w = load_w(w_cv)
            vca = p2.tile([P, NH, 66], bf16, tag="vv", bufs=1, name="vca")
            for ds in range(2):
                ps = psA.tile([T, 512], fp32, tag="mm", name="psvc")
                for kt in range(HT):
                    nc.tensor.matmul(ps[:], tg[:, kt, 0:T], w[:, kt, ds * 512:(ds + 1) * 512],
                                     start=(kt == 0), stop=(kt == HT - 1))
                nc.vector.tensor_copy(out=vca[0:T, ds * 8:(ds + 1) * 8, 0:64],
                                      in_=ps[:].rearrange("p (h c) -> p h c", c=64))
            nc.vector.memset(vca[0:T, :, 64:66], 1.0)

            # fused: per (qn, pair): q-proj -> scores -> exp -> PV -> normalize
            # (qn outer so LN1's qn=0 output unblocks cross work earliest)
            w = load_w(w_cq)
            ctxc = p1.tile([P, HT, QL], bf16, tag="ctx")
            for qn in range(QN):
                for a in range(HT):
                    qs = slice(qn * 512, (qn + 1) * 512)
                    ps = psA.tile([P, 512], fp32, tag="mm", name="psqc")
                    for kt in range(HT):
                        nc.tensor.matmul(ps[:], w[:, kt, a * P:(a + 1) * P],
                                         a_bf[:, kt, qs],
                                         start=(kt == 0), stop=(kt == HT - 1))
                    qc_t = p2.tile([P, 512], bf16, tag="ktmp", name="qc_t")
                    nc.scalar.activation(out=qc_t[:], in_=ps[:],
                                         func=mybir.ActivationFunctionType.Identity,
                                         bias=col(CQB8 + a), scale=0.125)
                    for hh in range(2):
                        sp = psA.tile([T, 512], fp32, tag="mm", name="spc")
                        nc.tensor.matmul(sp[:], kc[hh * 64:(hh + 1) * 64, a, 0:T],
                                         qc_t[hh * 64:(hh + 1) * 64, :],
                                         start=True, stop=True)
                        pt = p4.tile([T, 512], bf16, tag="ptile", bufs=4, name="ptc")
                        nc.scalar.activation(out=pt[:], in_=sp[:],
                                             func=mybir.ActivationFunctionType.Exp)
                        cp = psC.tile([65, 512], fp32, tag="ctx", name="cpc")
                        nc.tensor.matmul(cp[:], vca[0:T, 2 * a + hh, 0:65], pt[:],
                                         start=True, stop=True)
                        rec = p3.tile([1, 512], fp32, tag="rows", bufs=2, name="recc")
                        nc.vector.reciprocal(out=rec[:], in_=cp[64:65, :])
                        scr = dscr.tile([1, 512], fp32, name="scr_c")
                        nc.sync.dma_start(out=scr[:, :], in_=rec[:])
                        bc = p2.tile([64, 512], fp32, tag="bc64", name="bcc")
                        nc.sync.dma_start(out=bc[:], in_=scr[0:1, :].partition_broadcast(64))
                        nc.vector.tensor_mul(
                            out=ctxc[hh * 64:(hh + 1) * 64, a, qs],
                            in0=cp[0:64, :], in1=bc[:])

            # ---------- Phase 7: cross out-proj + residual + LN2 ----------
            w = load_w(w_co)
            proj_add_residual(w, ctxc, COB)
            c_bf = p1.tile([P, HT, QL], bf16, tag="act_bf")
            layer_norm(CLG, CLB, out_bf=c_bf)

            # ---------- Phase 8: FFN (chunk-outer: each weight block loads once) ----------
            for ch in range(FT // FC):
                inters = [p1.tile([P, FC, 512], bf16, tag="inter", bufs=2,
                                  name=f"inter{i}") for i in range(QN)]
                for mi in range(FC):
                    m = ch * FC + mi
                    wi = p3.tile([P, HT, P], bf16, tag="wi", bufs=2, name="wi")
                    nc.sync.dma_start(out=wi[:], in_=w_i[:, m * P:(m + 1) * P]
                                      .rearrange("(t p) c -> p t c", p=P))
                    for qn in range(QN):
                        qs = slice(qn * 512, (qn + 1) * 512)
                        ps = psA.tile([P, 512], fp32, tag="mm", name="psi")
                        for kt in range(HT):
                            nc.tensor.matmul(ps[:], wi[:, kt, :], c_bf[:, kt, qs],
                                             start=(kt == 0), stop=(kt == HT - 1))
                        nc.scalar.activation(out=inters[qn][:, mi, :], in_=ps[:],
                                             func=mybir.ActivationFunctionType.Gelu,
                                             bias=col(IB + m), scale=1.0)
                for mo in range(HT):
                    wo = p2.tile([P, FC, P], bf16, tag="wo", name="wo")
                    nc.sync.dma_start(out=wo[:], in_=w_o[ch * FC * P:(ch + 1) * FC * P,
                                                        mo * P:(mo + 1) * P]
                                      .rearrange("(t p) c -> p t c", p=P))
                    for qn in range(QN):
                        qs = slice(qn * 512, (qn + 1) * 512)
                        ps = psA.tile([P, 512], fp32, tag="mm", name="pso2")
                        for kt in range(FC):
                            nc.tensor.matmul(ps[:], wo[:, kt, :], inters[qn][:, kt, :],
                                             start=(kt == 0), stop=(kt == FC - 1))
                        if ch == 0:
                            nc.vector.scalar_tensor_tensor(
                                out=work[:, mo, qs], in0=ps[:], scalar=col(OB + mo),
                                in1=work[:, mo, qs],
                                op0=mybir.AluOpType.add, op1=mybir.AluOpType.add)
                        else:
                            nc.vector.tensor_add(out=work[:, mo, qs], in0=ps[:],
                                                 in1=work[:, mo, qs])

            layer_norm(OLG, OLB)
            for qn in range(QN):
                qs = slice(qn * 512, (qn + 1) * 512)
                for j in range(HT):
                    nc.sync.dma_start(out=yT[j * P:(j + 1) * P, qs],
                                      in_=work[:, j, qs])

    nc.compile()
    return nc, band_kq


def _get_program(er):
    key = int(er)
    if key not in _CACHE:
        _CACHE[key] = _build(key)
    return _CACHE[key]


def build_in_maps(inp, band_kq, er):
    x = inp["x"].astype(np.float32)
    B, S_, H_ = x.shape

    # host-side shared staging
    wcast = {n: inp[n].astype(np.float32).astype(BF)
             for n in ("sq_w", "sk_w", "sv_w", "so_w", "cq_w", "ck_w", "cv_w", "co_w",
                       "i_w", "o_w")}
    so_b_eff = inp["so_b"].astype(np.float32) + inp["sv_b"].astype(np.float32) @ inp["so_w"].astype(np.float32)
    co_b_eff = inp["co_b"].astype(np.float32) + inp["cv_b"].astype(np.float32) @ inp["co_w"].astype(np.float32)
    pvec = np.zeros((P, PCOLS), np.float32)

    def pack(colbase, vec):
        v = np.asarray(vec, np.float32).reshape(-1, P)  # [k, 128]
        pvec[:, colbase:colbase + v.shape[0]] = v.T

    pack(QB8, inp["sq_b"].astype(np.float32) * 0.125)
    pack(KB, inp["sk_b"])
    pack(SOB, so_b_eff)
    pack(SLG, inp["sln_g"]); pack(SLB, inp["sln_b"])
    pack(CQB8, inp["cq_b"].astype(np.float32) * 0.125)
    pack(CKB, inp["ck_b"])
    pack(COB, co_b_eff)
    pack(CLG, inp["cln_g"]); pack(CLB, inp["cln_b"])
    pack(IB, inp["i_b"])
    pack(OB, inp["o_b"])
    pack(OLG, inp["oln_g"]); pack(OLB, inp["oln_b"])
    pvec[:, ONECOL] = 1.0

    tags = inp["emb_table"].astype(np.float32)[np.asarray(inp["ent_ids"]).astype(np.int64)]  # [T, H]
    assert tags.shape[0] == 50, f"program compiled for 50 tags, got {tags.shape[0]}"
    tagsT = np.zeros((H, 64), BF)
    tagsT[:, :tags.shape[0]] = tags.T.astype(BF)
    ident = np.eye(P, dtype=BF)
    ones = np.ones((P, 1), BF)

    # band tiles in local (rotated) coords, per half: for half=1 the rotated
    # tail rows (k_rot >= S - QL) correspond to true keys k_rot - S.
    nb = max(len(band_kq), 1)
    band_h = np.zeros((2, nb, P, 512), BF)
    if band_kq and er > 0:
        for i, (kt, qn) in enumerate(band_kq):
            k_rot = kt * P + np.arange(P)[:, None]
            q_rot = qn * 512 + np.arange(512)[None, :]
            d = k_rot - q_rot
            m0 = np.abs(d) <= er
            m1 = np.where(k_rot >= S_ - QL, np.abs(d - S_) <= er, m0)
            band_h[0, i] = m0.astype(BF)
            band_h[1, i] = m1.astype(BF)

    in_maps = []
    for c in range(8):
        b, half = divmod(c, 2)
        xt = x[b].T  # [H, S]
        rot = np.concatenate([xt[:, half * QL:], xt[:, :half * QL]], axis=1)
        in_maps.append({
            "xT": np.ascontiguousarray(rot).astype(BF),
            "xh": np.ascontiguousarray(rot[:, :QL]),
            "w_sq": wcast["sq_w"], "w_sk": wcast["sk_w"], "w_sv": wcast["sv_w"],
            "w_so": wcast["so_w"], "w_cq": wcast["cq_w"], "w_ck": wcast["ck_w"],
            "w_cv": wcast["cv_w"], "w_co": wcast["co_w"],
            "w_i": wcast["i_w"], "w_o": wcast["o_w"],
            "pvec": pvec, "tagsT": tagsT, "ident": ident, "ones": ones,
            "band": np.ascontiguousarray(band_h[half]),
        })
    return in_maps


def kernel(**inputs):
    inp = {k: np.asarray(v) for k, v in inputs.items()}
    x = inp["x"]
    B, S_, H_ = x.shape
    er = int(inp["ent_range"])
    nc, band_kq = _get_program(er)
    in_maps = build_in_maps(inp, band_kq, er)

    res = run_bass_kernel_spmd(nc, in_maps, core_ids=list(range(8)))
    out = np.empty((B, S_, H_), np.float32)
    for c in range(8):
        b, half = divmod(c, 2)
        out[b, half * QL:(half + 1) * QL, :] = res.results[c]["yT"].T
    return out

